# revision 1
# baseline (speedup 1.0000x reference)
"""Causal self-attention (B=2, T=2048, C=768, H=12) on 8 TRN2 NeuronCores.

Sharding: core i handles batch b = i//4 and 3 consecutive heads h0 = 3*(i%4).
Each core produces a partial projection output [T, C] (sum over its 3 heads);
the host sums the 4 partials per batch and adds biases.

Per-core dataflow (all transposeless):
  - QK gen:  psum[128,512] = sum_ct Wqk[ct,h].T @ xT[ct]  -> rows 0:64 = Q^T
             (scale+bias folded), rows 64:128 = K^T.
  - V gen:   psum[128,192] = sum_ct xT[ct,tchunk].T @ Wv[ct] -> v in natural
             [T, hs] layout, stored per k-tile as [v | 1] (ones col -> denom).
  - Attn:    S^T tile = K_block @ Q^T  ([128 kpos, 512 q] in PSUM), exp on ACT
             (no max subtraction; scores are O(1)), causal mask multiply on
             diagonal tiles only, PV accumulates [y^T | denom] over k-tiles.
  - Norm:    recip(denom) broadcast across partitions via a K=1 matmul,
             y^T = y_unnorm^T * bcast.
  - Proj:    out[tchunk, :] = sum_h yT[h, tchunk].T @ Wp[h]  (PSUM -> DRAM).
"""

import os

os.environ.setdefault("MYCRO_LOCAL_CACHE", "1")

import numpy as np
import ml_dtypes

BF16_NP = ml_dtypes.bfloat16

import concourse.bass as bass
import concourse.bacc as bacc
import concourse.mybir as mybir
import concourse.tile as tile
from concourse.bass_utils import run_bass_kernel_spmd

T = 2048
C = 768
HS = 64
NH = 12
HPC = 3  # heads per core
NCORES = 8
CT = C // 128  # 6 contraction tiles for qkv/v gen
QC = 512  # q-chunk width
NQC = T // QC  # 4
NKT = T // 128  # 16 k-tiles (and T-chunks)
SCALE = 1.0 / 8.0  # 1/sqrt(HS)
F32 = mybir.dt.float32
BF16 = mybir.dt.bfloat16

_PROGRAM = None
DEBUG_DUMP = False


class _Bacc(bacc.Bacc):
    # Pin the ACT function-table set: Exp and Ln both live in
    # natural_log_exp_and_others, but the default greedy chooser alternates
    # between exp_and_others and the ln set (25 x 1.3us reloads).
    def insert_act_table_loads(self):
        import bass_rust as _br
        from concourse.hw_specs import get_activation_tables

        has_activation = any(
            isinstance(i, mybir.InstActivation)
            for b in self.main_func.blocks
            for i in b.instructions
        )
        if not has_activation:
            return
        tables = sorted(
            get_activation_tables(self.m.arch).items(),
            key=lambda kv: kv[0] != "natural_log_exp_and_others",
        )
        _br.insert_act_table_loads(self, tables)


def _build_program():
    nc = bacc.Bacc("TRN2")
    xT_d = nc.declare_dram_parameter("xT", [128, CT, T], BF16, isOutput=False)
    wqk_d = nc.declare_dram_parameter("wqk", [128, CT, HPC, 128], BF16, isOutput=False)
    wv_d = nc.declare_dram_parameter("wv", [128, CT, HPC * HS], BF16, isOutput=False)
    wp_d = nc.declare_dram_parameter("wp", [128, HPC, C], BF16, isOutput=False)
    bq_d = nc.declare_dram_parameter("bq", [HS, HPC], F32, isOutput=False)
    mask_d = nc.declare_dram_parameter("mask", [128, QC], BF16, isOutput=False)
    out_d = nc.declare_dram_parameter("out", [T, C], F32, isOutput=True)
    dbg = {}
    if DEBUG_DUMP:
        for _n in ("dbg_q", "dbg_k", "dbg_y"):
            dbg[_n] = nc.declare_dram_parameter(_n, [HS, HPC, T], F32, isOutput=True)

    with tile.TileContext(nc) as tc:
        with (
            tc.tile_pool(name="const", bufs=1) as constp,
            tc.tile_pool(name="big", bufs=1) as bigp,
            tc.tile_pool(name="exps", bufs=8) as expp,
            tc.tile_pool(name="work", bufs=3) as workp,
            tc.tile_pool(name="ps_s", bufs=1, space="PSUM") as ps_s,
            tc.tile_pool(name="ps_y", bufs=1, space="PSUM") as ps_y,
            tc.tile_pool(name="ps_m", bufs=2, space="PSUM") as ps_m,
        ):
            # ---- clock-governor warm-up: the HAM governor needs ~3us of
            # continuous PE busy to lift the clock 1.2 -> 2.4 GHz; burn
            # matmuls on memset tiles while the input DMA lands so gen runs
            # at full clock (v1 ran all of gen at half clock).
            warm_a = constp.tile([128, 128], BF16, name="warm_a")
            warm_b = constp.tile([128, QC], BF16, name="warm_b")
            nc.vector.memset(warm_a, 0.125)
            nc.vector.memset(warm_b, 0.125)
            for i in range(14):
                pw = ps_m.tile([128, QC], F32, tag="misc", name=f"warm{i}")
                nc.tensor.matmul(pw, warm_a, warm_b, start=True, stop=True)

            # ---- inputs -> SBUF; priority order: wqk + bq + the first
            # 512-col chunk of every xT c-tile (feeds qkgen(0) jq0), then
            # wv + mask on sync; later xT chunks + wp go via the scalar
            # engine's DGE so arrival tracks PE consumption order.
            wqk = constp.tile([128, CT, HPC, 128], BF16)
            nc.sync.dma_start(out=wqk, in_=wqk_d[:])
            bq = constp.tile([HS, HPC], F32)
            nc.sync.dma_start(out=bq, in_=bq_d[:])
            xTs = []
            for ct in range(CT):
                x1 = bigp.tile([128, T], BF16, name=f"xT{ct}")
                nc.sync.dma_start(out=x1[:, 0:QC], in_=xT_d[:, ct, 0:QC])
                xTs.append(x1)
            wv = constp.tile([128, CT, HPC * HS], BF16)
            nc.sync.dma_start(out=wv, in_=wv_d[:])
            mask = constp.tile([128, QC], BF16)
            nc.sync.dma_start(out=mask, in_=mask_d[:])
            for ct in range(CT):
                nc.scalar.dma_start(
                    out=xTs[ct][:, QC : 2 * QC], in_=xT_d[:, ct, QC : 2 * QC]
                )
            for ct in range(CT):
                nc.scalar.dma_start(
                    out=xTs[ct][:, 2 * QC : T], in_=xT_d[:, ct, 2 * QC : T]
                )
            wp = constp.tile([128, HPC, C], BF16)
            nc.scalar.dma_start(out=wp, in_=wp_d[:])
            ones = constp.tile([128, HS], BF16)
            nc.vector.memset(ones, 1.0)

            qT = bigp.tile([128, HPC, T], BF16)
            kT = bigp.tile([128, HPC, T], BF16)
            vsb = bigp.tile([128, NKT, HPC, HS + 1], BF16)  # [v | 1] per head
            yT = bigp.tile([128, HPC, T], BF16)
            yun = bigp.tile([128, HPC, T], F32)  # unnorm y^T; row 64 = denom
            rcp = bigp.tile([128, HPC, T], BF16)  # 1/denominators, row 64

            nc.gpsimd.memset(vsb[:, :, :, HS], 1.0)
            # zero the bottom halves: K=64 contractions are padded to K=128
            # (zero rows are numerically free) because half-array row-group
            # matmuls do not count as PE-busy for the HAM clock governor --
            # with K=64 the whole attention phase runs at 1.2 GHz (K=4/8).
            # qT/kT are needed first; yun/yT zero-fills are emitted after
            # QKV-gen so the 5us DVE memsets don't block the evacuations.
            nc.vector.memset(qT[HS:128, :, :], 0.0)
            nc.gpsimd.memset(kT[HS:128, :, :], 0.0)

            # ---- QK^T generation (head 0 first so attention can start early)
            def qkgen(h):
                for jq in range(NQC):
                    pqk = ps_m.tile([128, QC], F32, tag="misc")
                    for ct in range(CT):
                        nc.tensor.matmul(
                            pqk,
                            wqk[:, ct, h, :],
                            xTs[ct][:, jq * QC : (jq + 1) * QC],
                            start=(ct == 0),
                            stop=(ct == CT - 1),
                        )
                    nc.vector.tensor_scalar_add(
                        qT[0:HS, h, jq * QC : (jq + 1) * QC],
                        pqk[0:HS, :],
                        bq[:, h : h + 1],
                    )
                    # partition-shifting evacuation (64:128 -> 0:64); legal on
                    # DVE at 64 channels (bank0->Q0, bank1->Q1, reads follow
                    # the src access pattern)
                    nc.vector.tensor_copy(
                        kT[0:HS, h, jq * QC : (jq + 1) * QC], pqk[64:128, :]
                    )

            qkgen(0)

            # ---- V generation (natural [T, hs] layout, + ones column)
            for m in range(NKT):
                pv = ps_m.tile([128, QC], F32, tag="misc")
                for ct in range(CT):
                    nc.tensor.matmul(
                        pv[:, 0 : HPC * HS],
                        xTs[ct][:, m * 128 : (m + 1) * 128],
                        wv[:, ct, :],
                        start=(ct == 0),
                        stop=(ct == CT - 1),
                    )
                nc.vector.tensor_copy(
                    vsb[:, m, :, 0:HS],
                    pv[:, 0 : HPC * HS].rearrange("p (h d) -> p h d", h=HPC),
                )

            qkgen(1)
            qkgen(2)
            nc.gpsimd.memset(yun[HS:128, :, :], 0.0)
            nc.gpsimd.memset(yT[HS:128, :, :], 0.0)

            # ---- attention: interleave pairs of independent (jq, h)
            # units so the PE streams one unit's matmuls while ACT runs the
            # other's exp (keeps the PE dense -> HAM stays at 2.4 GHz)

            def tile_geom(jq, kt):
                if kt < 4 * jq:  # full k-tile
                    return QC, 0
                r = kt - 4 * jq
                return QC - 128 * r, 128 * r

            def s_mms(es_p, jq, h, g):
                q0 = jq * QC
                for s in range(2):
                    kt = 2 * g + s
                    w, qoff = tile_geom(jq, kt)
                    nc.tensor.matmul(
                        es_p[:, s * QC : s * QC + w],
                        kT[:, h, kt * 128 : (kt + 1) * 128],
                        qT[:, h, q0 + qoff : q0 + QC],
                        start=True,
                        stop=True,
                    )

            def exp_mask(es_p, es_b, jq, g):
                kt0 = 2 * g
                if kt0 + 1 < 4 * jq:  # both full
                    nc.scalar.activation(
                        es_b[:, 0 : 2 * QC],
                        es_p[:, 0 : 2 * QC],
                        mybir.ActivationFunctionType.Exp,
                    )
                else:
                    r0 = kt0 - 4 * jq  # 0 or 2
                    if r0 == 0:  # widths 512, 384: one contiguous span
                        nc.scalar.activation(
                            es_b[:, 0 : QC + 384],
                            es_p[:, 0 : QC + 384],
                            mybir.ActivationFunctionType.Exp,
                        )
                    else:  # widths 256, 128: two disjoint spans
                        nc.scalar.activation(
                            es_b[:, 0:256],
                            es_p[:, 0:256],
                            mybir.ActivationFunctionType.Exp,
                        )
                        nc.scalar.activation(
                            es_b[:, QC : QC + 128],
                            es_p[:, QC : QC + 128],
                            mybir.ActivationFunctionType.Exp,
                        )
                    for s in range(2):
                        w = QC - 128 * (kt0 + s - 4 * jq)
                        nc.vector.tensor_mul(
                            es_b[:, s * QC : s * QC + w],
                            es_b[:, s * QC : s * QC + w],
                            mask[:, 0:w],
                        )

            def pv_mms(py, es_b, jq, h, g):
                for s in range(2):
                    kt = 2 * g + s
                    w, qoff = tile_geom(jq, kt)
                    nc.tensor.matmul(
                        py[0 : HS + 1, qoff:QC],
                        vsb[:, kt, h, :],
                        es_b[:, s * QC : s * QC + w],
                        start=(kt == 0),
                        stop=(kt == 4 * jq + 3),
                        skip_group_check=True,
                    )

            def recip_cols_ops(c0, c1):
                # 1/d = exp(-ln d) on ACT over columns [c0:c1] for all heads
                return [
                    lambda: nc.scalar.activation(
                        rcp[64:65, :, c0:c1],
                        yun[64:65, :, c0:c1],
                        mybir.ActivationFunctionType.Ln,
                    ),
                    lambda: nc.scalar.activation(
                        rcp[64:65, :, c0:c1],
                        rcp[64:65, :, c0:c1],
                        mybir.ActivationFunctionType.Exp,
                        scale=-1.0,
                    ),
                ]

            def normalize(jq, h):
                q0 = jq * QC
                pb = ps_y.tile(
                    [128, QC], F32, tag=f"py{(jq * HPC + h) % 2}", name=f"pb{jq}_{h}"
                )
                nc.tensor.matmul(
                    pb[0:HS, :],
                    ones[64:65, 0:HS],
                    rcp[64:65, h, q0 : q0 + QC],
                    start=True,
                    stop=True,
                )
                bc = workp.tile([128, QC], F32, tag="bc", name=f"bc{jq}_{h}")
                nc.vector.tensor_copy(bc[0:HS, :], pb[0:HS, :])
                nc.vector.tensor_mul(
                    yT[0:HS, h, q0 : q0 + QC],
                    yun[0:HS, h, q0 : q0 + QC],
                    bc[0:HS, :],
                )

            def proj(t):
                ob = workp.tile([128, C], F32, tag="ob", name=f"ob{t}")
                for n0, w in ((0, 512), (512, 256)):
                    po = ps_s.tile(
                        [128, QC],
                        F32,
                        tag=f"es{(2 * t + (n0 > 0)) % 2}",
                        name=f"po{t}_{n0}",
                    )
                    for h in range(HPC):
                        nc.tensor.matmul(
                            po[:, 0:w],
                            yT[:, h, t * 128 : (t + 1) * 128],
                            wp[:, h, n0 : n0 + w],
                            start=(h == 0),
                            stop=(h == HPC - 1),
                        )
                    if w == 512 and t % 2 == 0:
                        nc.scalar.copy(ob[:, n0 : n0 + w], po[:, 0:w])
                    else:
                        nc.vector.tensor_copy(ob[:, n0 : n0 + w], po[:, 0:w])
                nc.sync.dma_start(out=out_d[t * 128 : (t + 1) * 128, :], in_=ob)

            # h-major unit order: after pair 4, q-chunks 0-1 are complete for
            # every head, so their reciprocal + normalize + projection overlap
            # the last attention pair instead of serializing at the end
            # pair q-chunks with adjacent group counts ((2,3) then (0,1))
            # and put the short pair last: after pair 4, q-chunks 2-3 are
            # complete for every head, so their reciprocal runs during the
            # short last pair with a minimal ACT-pipeline stall
            units = [(jq, h) for h in range(HPC) for jq in range(NQC)]
            for pi in range(0, len(units), 2):
                if pi == 10:
                    for op in recip_cols_ops(0, 2 * QC):
                        op()
                lanes = []
                for li, (jq, h) in enumerate(units[pi : pi + 2]):
                    lanes.append(
                        {
                            "jq": jq,
                            "h": h,
                            "G": 2 * jq + 2,
                            "py": ps_y.tile([128, QC], F32, tag=f"py{li}", name=f"py{li}_{pi}"),
                            "li": li,
                            "ebs": {},
                        }
                    )
                max_g = max(ln["G"] for ln in lanes)
                for g in range(max_g + 1):  # PV lags S by one round
                    for ln in lanes:
                        if g < ln["G"]:
                            es_p = ps_s.tile(
                                [128, 2 * QC],
                                F32,
                                tag=f"es{ln['li']}",
                                name=f"es{ln['li']}_{pi}_{g}",
                            )
                            es_b = expp.tile([128, 2 * QC], BF16, tag="ex")
                            ln["ebs"][g] = es_b
                            s_mms(es_p, ln["jq"], ln["h"], g)
                            exp_mask(es_p, es_b, ln["jq"], g)
                        if 0 <= g - 1 < ln["G"]:
                            pv_mms(
                                ln["py"],
                                ln["ebs"].pop(g - 1),
                                ln["jq"],
                                ln["h"],
                                g - 1,
                            )
                for ln in lanes:
                    jq, h, py = ln["jq"], ln["h"], ln["py"]
                    q0 = jq * QC
                    # stash unnormalized y + denominator row; frees the PSUM
                    # slot without putting per-chunk Ln/Exp on ACT
                    nc.vector.tensor_copy(
                        yun[0 : HS + 1, h, q0 : q0 + QC], py[0 : HS + 1, :]
                    )

            # ---- tail: q-chunks 0/3 were reciprocal'd during the last
            # pair, so their normalize + projection overlaps the reciprocal
            # of q-chunks 1/2 on ACT
            for op in recip_cols_ops(2 * QC, T):
                op()
            for jq in (0, 1):
                for h in range(HPC):
                    normalize(jq, h)
            for t in range(8):
                proj(t)
            for jq in (2, 3):
                for h in range(HPC):
                    normalize(jq, h)
            for t in range(8, NKT):
                proj(t)
            if DEBUG_DUMP:
                for name, tl in (("dbg_q", qT), ("dbg_k", kT), ("dbg_y", yT)):
                    st = workp.tile([HS, HPC, T], F32, tag="dbgst")
                    nc.vector.tensor_copy(st, tl)
                    nc.sync.dma_start(out=dbg[name][:], in_=st)
    return nc


def get_program():
    global _PROGRAM
    if _PROGRAM is None:
        _PROGRAM = _build_program()
        if not _PROGRAM.is_finalized():
            _PROGRAM.finalize()
    return _PROGRAM


def make_in_maps(x, W_attn, b_attn):
    x = np.asarray(x, dtype=np.float32)
    W_attn = np.asarray(W_attn, dtype=np.float32)
    b_attn = np.asarray(b_attn, dtype=np.float32)
    mask_arr = (
        np.arange(128, dtype=np.int64)[:, None] <= np.arange(QC, dtype=np.int64)[None, :]
    ).astype(BF16_NP)
    in_maps = []
    for i in range(NCORES):
        b = i // 4
        h0 = HPC * (i % 4)
        xb = x[b]  # [T, C]
        xT_arr = np.ascontiguousarray(
            xb.T.reshape(CT, 128, T).transpose(1, 0, 2)
        ).astype(BF16_NP)  # [p, ct, t]
        Wq = (
            W_attn[:, h0 * HS : (h0 + HPC) * HS].reshape(C, HPC, HS) * SCALE
        )
        Wk = W_attn[:, C + h0 * HS : C + (h0 + HPC) * HS].reshape(C, HPC, HS)
        wqk_full = np.concatenate([Wq, Wk], axis=2)  # [C, HPC, 128]
        wqk_arr = np.ascontiguousarray(
            wqk_full.reshape(CT, 128, HPC, 128).transpose(1, 0, 2, 3)
        ).astype(BF16_NP)
        wv_arr = np.ascontiguousarray(
            W_attn[:, 2 * C + h0 * HS : 2 * C + (h0 + HPC) * HS]
            .reshape(CT, 128, HPC * HS)
            .transpose(1, 0, 2)
        ).astype(BF16_NP)
        bq_arr = np.ascontiguousarray(
            (b_attn[h0 * HS : (h0 + HPC) * HS] * SCALE).reshape(HPC, HS).T
        )
        in_maps.append(
            {
                "xT": xT_arr,
                "wqk": wqk_arr,
                "wv": wv_arr,
                "bq": bq_arr,
                "mask": mask_arr,
            }
        )
    return in_maps


def add_wp(in_maps, W_proj):
    W_proj = np.asarray(W_proj, dtype=np.float32)
    for i in range(NCORES):
        h0 = HPC * (i % 4)
        wp_arr = np.zeros((128, HPC, C), dtype=BF16_NP)
        wp_arr[:HS] = (
            W_proj[h0 * HS : (h0 + HPC) * HS, :]
            .reshape(HPC, HS, C)
            .transpose(1, 0, 2)
            .astype(BF16_NP)
        )
        in_maps[i]["wp"] = wp_arr
    return in_maps


def gather(results, b_attn, W_proj, b_proj):
    b_attn = np.asarray(b_attn, dtype=np.float32)
    W_proj = np.asarray(W_proj, dtype=np.float32)
    b_proj = np.asarray(b_proj, dtype=np.float32)
    parts = [np.asarray(r["out"], dtype=np.float32) for r in results]
    out = np.stack(
        [parts[0] + parts[1] + parts[2] + parts[3], parts[4] + parts[5] + parts[6] + parts[7]]
    )
    # b_v adds to y after normalization -> constant vector through the proj.
    # b_k provably cancels in softmax; b_q is handled on-device.
    const = b_proj + b_attn[2 * C : 3 * C] @ W_proj
    return out + const[None, None, :]


def run(x, W_attn, b_attn, W_proj, b_proj, trace=False):
    nc = get_program()
    in_maps = add_wp(make_in_maps(x, W_attn, b_attn), W_proj)
    res = run_bass_kernel_spmd(nc, in_maps, list(range(NCORES)), trace=trace)
    out = gather(res.results, b_attn, W_proj, b_proj)
    return out, res


def kernel(x, W_attn, b_attn, W_proj, b_proj):
    out, _ = run(x, W_attn, b_attn, W_proj, b_proj, trace=False)
    return out

